# revision 6
# baseline (speedup 1.0000x reference)
"""NeuSum topk_masking kernel for 8x TRN2 NeuronCores (Bass/Tile).

Key observation: in the reference, h starts as h0 broadcast over doc_len and
every GRU update depends only on per-document quantities (s_prev is a
broadcast of one selected sentence vector), so h never varies across the L
axis.  The whole recurrence therefore runs on (B, EH) vectors; the only
O(B*L) work is d_proj = sent_enc @ wd.T (loop-invariant) and, per iteration,
score = ws . tanh(q_b + d_proj) + bs, followed by a per-document argmax,
mask update and a 1-row gather.

Sharding: data-parallel over batch, 16 docs per core, weights replicated.

Layout: "T layout" (feature on partitions, doc/position on free dim)
everywhere, obtained by host-side pre-transposition of the inputs.  The
per-doc q vector is then a per-partition column, which fuses into the ACT
tanh as its bias operand.
"""

import os
import sys

import numpy as np

sys.path.insert(0, "/opt/trn_rl_repo")

import concourse.bass as bass
import concourse.mybir as mybir
import concourse.tile as tile
from concourse import bacc, bass_utils
from concourse.bass import IndirectOffsetOnAxis
from concourse.masks import make_identity

F32 = mybir.dt.float32
F32R = mybir.dt.float32r
I32 = mybir.dt.int32
U32 = mybir.dt.uint32
U8 = mybir.dt.uint8

B, L, SH, EH, A, T = 128, 200, 512, 512, 512, 3
D2 = 2 * SH          # 1024
NCORES = 8
D = B // NCORES      # 16 docs per core
ML = D * L           # 3200 rows per core
NEG = -1.0e6

# Per-matmul-site fp32r (full-rate) switches; fp32 (4 cyc/col) when False.
USE_F32R_DPROJ = os.environ.get("K_F32R_DPROJ", "0") == "1"
USE_F32R_WSDOT = os.environ.get("K_F32R_WSDOT", "0") == "1"
USE_F32R_GRU = os.environ.get("K_F32R_GRU", "0") == "1"

DP_BLK = 256                 # d_proj ml-block width (psum free dim)
SC_CH = 2 * L                # score chunk = 2 docs = 400 cols


def _r(ap, use_r):
    return ap.bitcast(F32R) if use_r else ap


def build_program():
    nc = bacc.Bacc(
        "TRN2",
        target_bir_lowering=False,
        debug=False,
        num_devices=NCORES,
    )

    # ---- DRAM tensors (per-core shapes; SPMD over 8 cores) ----
    d_xT = nc.dram_tensor("xT", [D2, ML], F32, kind="ExternalInput").ap()
    d_xnat = nc.dram_tensor("xnat", [ML, D2], F32, kind="ExternalInput").ap()
    d_wdT = nc.dram_tensor("wdT", [D2, A], F32, kind="ExternalInput").ap()
    d_wihT = nc.dram_tensor("wihT", [D2, 3 * EH], F32, kind="ExternalInput").ap()
    d_whhT = nc.dram_tensor("whhT", [EH, 3 * EH], F32, kind="ExternalInput").ap()
    d_wqT = nc.dram_tensor("wqT", [EH, A], F32, kind="ExternalInput").ap()
    d_h0pk = nc.dram_tensor("h0pk", [128, 4 * D], F32, kind="ExternalInput").ap()
    d_qb = nc.dram_tensor("qb", [128, 4], F32, kind="ExternalInput").ap()
    d_brz = nc.dram_tensor("brz", [128, 8 * D], F32, kind="ExternalInput").ap()
    d_bhn = nc.dram_tensor("bhn", [128, 4 * D], F32, kind="ExternalInput").ap()
    d_bxn = nc.dram_tensor("bxn", [128, 4 * D], F32, kind="ExternalInput").ap()
    d_wsk = nc.dram_tensor("wsk", [128, 4], F32, kind="ExternalInput").ap()
    d_bs = nc.dram_tensor("bs", [1, 1], F32, kind="ExternalInput").ap()
    d_iota = nc.dram_tensor("iota", [D, L], F32, kind="ExternalInput").ap()
    d_docoff = nc.dram_tensor("docoff", [D, 1], I32, kind="ExternalInput").ap()

    d_scores = nc.dram_tensor("scores_out", [D, T, L], F32, kind="ExternalOutput").ap()
    d_sel = nc.dram_tensor("sel_out", [D, T], I32, kind="ExternalOutput").ap()

    KT = D2 // 128   # 8 k-tiles of contraction over 2*SH
    KH = EH // 128   # 4 k-tiles of contraction over EH
    AT = A // 128    # 4 a-tiles

    with tile.TileContext(nc) as tc:
        # ---------------- persistent pools ----------------
        with (
            tc.tile_pool(name="weights", bufs=1) as wpool,
            tc.tile_pool(name="dproj", bufs=1) as dpool,
            tc.tile_pool(name="small", bufs=1) as spool,
            tc.tile_pool(name="state", bufs=2) as stpool,
            tc.tile_pool(name="stage", bufs=2) as stagepool,
            tc.tile_pool(name="th", bufs=2) as thpool,
            tc.tile_pool(name="dpps", bufs=4, space="PSUM") as dpps,
            tc.tile_pool(name="scps", bufs=2, space="PSUM") as scps,
            tc.tile_pool(name="smps", bufs=2, space="PSUM") as smps,
        ):
            # ---- load early weights / constants ----
            wdT_sb = wpool.tile([128, KT * A], F32, tag="wdT")
            nc.sync.dma_start(
                out=wdT_sb[:].rearrange("p (k j) -> p k j", j=A),
                in_=d_wdT.rearrange("(k p) j -> p k j", p=128),
            )
            whhT_sb = wpool.tile([128, KH * 3 * EH], F32, tag="whhT")
            nc.sync.dma_start(
                out=whhT_sb[:].rearrange("p (k j) -> p k j", j=3 * EH),
                in_=d_whhT.rearrange("(k p) j -> p k j", p=128),
            )
            wqT_sb = wpool.tile([128, KH * A], F32, tag="wqT")
            nc.sync.dma_start(
                out=wqT_sb[:].rearrange("p (k j) -> p k j", j=A),
                in_=d_wqT.rearrange("(k p) j -> p k j", p=128),
            )

            h0pk_sb = spool.tile([128, 4 * D], F32, tag="h0pk")
            nc.sync.dma_start(out=h0pk_sb[:], in_=d_h0pk)
            qb_sb = spool.tile([128, 4], F32, tag="qb")
            nc.sync.dma_start(out=qb_sb[:], in_=d_qb)
            brz_sb = spool.tile([128, 8 * D], F32, tag="brz")
            nc.sync.dma_start(out=brz_sb[:], in_=d_brz)
            bhn_sb = spool.tile([128, 4 * D], F32, tag="bhn")
            nc.sync.dma_start(out=bhn_sb[:], in_=d_bhn)
            bxn_sb = spool.tile([128, 4 * D], F32, tag="bxn")
            nc.sync.dma_start(out=bxn_sb[:], in_=d_bxn)
            wsk_sb = spool.tile([128, 4], F32, tag="wsk")
            nc.sync.dma_start(out=wsk_sb[:], in_=d_wsk)
            bs_sb = spool.tile([1, 1], F32, tag="bs")
            nc.sync.dma_start(out=bs_sb[:], in_=d_bs)
            iota_sb = spool.tile([D, L], F32, tag="iota")
            nc.sync.dma_start(out=iota_sb[:], in_=d_iota)
            docoff_sb = spool.tile([D, 1], I32, tag="docoff")
            nc.sync.dma_start(out=docoff_sb[:], in_=d_docoff)

            ident = spool.tile([128, 128], F32, tag="ident")
            make_identity(nc, ident[:])

            negt = spool.tile([D, L], F32, tag="negt")
            nc.vector.memset(negt[:], NEG)
            accmask = stpool.tile([D, L], U8, tag="accmask")
            nc.vector.memset(accmask[:], 0.0)

            # ---- d_proj: dprojT[a][p, ml] = sum_k wd[128a+p, :] . x[ml, :] ----
            dprojT = [dpool.tile([128, ML], F32, tag=f"dp{a}", name=f"dp{a}")
                      for a in range(AT)]

            n_blk = (ML + DP_BLK - 1) // DP_BLK
            with tc.tile_pool(name="xt", bufs=16) as xtpool:
                for blk in range(n_blk):
                    c0 = blk * DP_BLK
                    w = min(DP_BLK, ML - c0)
                    xts = []
                    for k in range(KT):
                        xt = xtpool.tile([128, DP_BLK], F32, tag="xt")
                        nc.sync.dma_start(
                            out=xt[:, :w],
                            in_=d_xT[k * 128:(k + 1) * 128, c0:c0 + w],
                        )
                        xts.append(xt)
                    for a in range(AT):
                        ps = dpps.tile([128, DP_BLK], F32, tag="dpps")
                        for k in range(KT):
                            nc.tensor.matmul(
                                out=ps[:, :w],
                                lhsT=_r(wdT_sb[:, k * A + a * 128:k * A + (a + 1) * 128],
                                        USE_F32R_DPROJ),
                                rhs=_r(xts[k][:, :w], USE_F32R_DPROJ),
                                start=(k == 0),
                                stop=(k == KT - 1),
                            )
                        nc.vector.tensor_copy(dprojT[a][:, c0:c0 + w], ps[:, :w])

            # wihT is only needed from t=1; let its DMA land after the x stream.
            wihT_sb = wpool.tile([128, KT * 3 * EH], F32, tag="wihT")
            nc.sync.dma_start(
                out=wihT_sb[:].rearrange("p (k j) -> p k j", j=3 * EH),
                in_=d_wihT.rearrange("(k p) j -> p k j", p=128),
            )

            # ---------------- T loop ----------------
            hpk = h0pk_sb          # (128, 4*D): h[b, 128a+p] at col a*D+b
            flat_idx = None

            for t in range(T):
                # -- ghT: gh = h @ whh.T, natural (D, 3EH) then packed (128, 12*D)
                ghpk = stpool.tile([128, 12 * D], F32, tag="ghpk")
                for nch in range(3):
                    psg = smps.tile([D, 512], F32, tag="smps")
                    for k in range(KH):
                        nc.tensor.matmul(
                            out=psg[:],
                            lhsT=_r(hpk[:, k * D:(k + 1) * D], USE_F32R_GRU),
                            rhs=_r(whhT_sb[:, k * 3 * EH + nch * 512:
                                           k * 3 * EH + (nch + 1) * 512], USE_F32R_GRU),
                            start=(k == 0),
                            stop=(k == KH - 1),
                        )
                    gst = stagepool.tile([D, 512], F32, tag="gst")
                    nc.vector.tensor_copy(gst[:], psg[:])
                    for jj in range(4):
                        jt = nch * 4 + jj
                        pst = smps.tile([128, D], F32, tag="smps")
                        nc.tensor.transpose(
                            out=pst[:], in_=gst[:, jj * 128:(jj + 1) * 128],
                            identity=ident[:D, :D],
                        )
                        nc.vector.tensor_copy(ghpk[:, jt * D:(jt + 1) * D], pst[:])

                gxpk = None
                if t > 0:
                    # -- gather previously selected sentence rows
                    ssel = stagepool.tile([D, D2], F32, tag="ssel", bufs=1)
                    nc.gpsimd.indirect_dma_start(
                        out=ssel[:],
                        out_offset=None,
                        in_=d_xnat,
                        in_offset=IndirectOffsetOnAxis(ap=flat_idx[:, :1], axis=0),
                    )
                    # s_sel.T packed: (128, KT*D)
                    sspk = stpool.tile([128, KT * D], F32, tag="sspk")
                    for k in range(KT):
                        pst = smps.tile([128, D], F32, tag="smps")
                        nc.tensor.transpose(
                            out=pst[:], in_=ssel[:, k * 128:(k + 1) * 128],
                            identity=ident[:D, :D],
                        )
                        nc.vector.tensor_copy(sspk[:, k * D:(k + 1) * D], pst[:])
                    # -- gxT: gx = s_sel @ wih.T
                    gxpk = stpool.tile([128, 12 * D], F32, tag="gxpk")
                    for nch in range(3):
                        psg = smps.tile([D, 512], F32, tag="smps")
                        for k in range(KT):
                            nc.tensor.matmul(
                                out=psg[:],
                                lhsT=_r(sspk[:, k * D:(k + 1) * D], USE_F32R_GRU),
                                rhs=_r(wihT_sb[:, k * 3 * EH + nch * 512:
                                               k * 3 * EH + (nch + 1) * 512],
                                       USE_F32R_GRU),
                                start=(k == 0),
                                stop=(k == KT - 1),
                            )
                        gst = stagepool.tile([D, 512], F32, tag="gst")
                        nc.vector.tensor_copy(gst[:], psg[:])
                        for jj in range(4):
                            jt = nch * 4 + jj
                            pst = smps.tile([128, D], F32, tag="smps")
                            nc.tensor.transpose(
                                out=pst[:], in_=gst[:, jj * 128:(jj + 1) * 128],
                                identity=ident[:D, :D],
                            )
                            nc.vector.tensor_copy(gxpk[:, jt * D:(jt + 1) * D], pst[:])

                # -- gates in packed T layout --
                # r, z: cols [0, 8D) of g*pk; n: cols [8D, 12D)
                srz = stagepool.tile([128, 8 * D], F32, tag="srz")
                if t > 0:
                    nc.vector.tensor_add(srz[:], gxpk[:, :8 * D], ghpk[:, :8 * D])
                    nc.vector.tensor_add(srz[:], srz[:], brz_sb[:])
                else:
                    nc.vector.tensor_add(srz[:], ghpk[:, :8 * D], brz_sb[:])
                rz = stagepool.tile([128, 8 * D], F32, tag="rz")
                nc.scalar.activation(rz[:], srz[:], mybir.ActivationFunctionType.Sigmoid)

                hn = stagepool.tile([128, 4 * D], F32, tag="hn")
                nc.vector.tensor_add(hn[:], ghpk[:, 8 * D:12 * D], bhn_sb[:])
                xn = stagepool.tile([128, 4 * D], F32, tag="xn")
                if t > 0:
                    nc.vector.tensor_add(xn[:], gxpk[:, 8 * D:12 * D], bxn_sb[:])
                else:
                    nc.vector.tensor_copy(xn[:], bxn_sb[:])
                u = stagepool.tile([128, 4 * D], F32, tag="u")
                nc.vector.tensor_mul(u[:], rz[:, :4 * D], hn[:])
                nc.vector.tensor_add(u[:], u[:], xn[:])
                nt = stagepool.tile([128, 4 * D], F32, tag="nt")
                nc.scalar.activation(nt[:], u[:], mybir.ActivationFunctionType.Tanh)

                dd = stagepool.tile([128, 4 * D], F32, tag="dd")
                nc.vector.tensor_sub(dd[:], hpk[:], nt[:])
                nc.vector.tensor_mul(dd[:], rz[:, 4 * D:8 * D], dd[:])
                hnew = stpool.tile([128, 4 * D], F32, tag="h")
                nc.vector.tensor_add(hnew[:], dd[:], nt[:])
                hpk = hnew

                # -- q = h @ wq.T + (bq + bd), packed (128, 4*D) --
                psq = smps.tile([D, A], F32, tag="smps")
                for k in range(KH):
                    nc.tensor.matmul(
                        out=psq[:],
                        lhsT=_r(hpk[:, k * D:(k + 1) * D], USE_F32R_GRU),
                        rhs=_r(wqT_sb[:, k * A:(k + 1) * A], USE_F32R_GRU),
                        start=(k == 0),
                        stop=(k == KH - 1),
                    )
                qst = stagepool.tile([D, A], F32, tag="qst", bufs=1)
                nc.vector.tensor_copy(qst[:], psq[:])
                qpk = stpool.tile([128, 4 * D], F32, tag="qpk")
                for a in range(AT):
                    pst = smps.tile([128, D], F32, tag="smps")
                    nc.tensor.transpose(
                        out=pst[:], in_=qst[:, a * 128:(a + 1) * 128],
                        identity=ident[:D, :D],
                    )
                    nc.scalar.activation(
                        qpk[:, a * D:(a + 1) * D], pst[:],
                        mybir.ActivationFunctionType.Identity,
                        bias=qb_sb[:, a:a + 1],
                    )

                # -- scores: ws . tanh(q_b + d_proj) + bs  --
                sc16 = stpool.tile([D, L], F32, tag="sc16")
                n_ch = ML // SC_CH      # 8 chunks of 2 docs
                for ch in range(n_ch):
                    ths = [thpool.tile([128, SC_CH], F32, tag=f"th{a}", name=f"th{a}_{t}_{ch}")
                           for a in range(AT)]
                    for a in range(AT):
                        for bb in range(2):
                            b = 2 * ch + bb
                            nc.scalar.activation(
                                ths[a][:, bb * L:(bb + 1) * L],
                                dprojT[a][:, b * L:(b + 1) * L],
                                mybir.ActivationFunctionType.Tanh,
                                bias=qpk[:, a * D + b:a * D + b + 1],
                            )
                    pssc = scps.tile([1, SC_CH], F32, tag="scps")
                    for a in range(AT):
                        nc.tensor.matmul(
                            out=pssc[:],
                            lhsT=_r(wsk_sb[:, a:a + 1], USE_F32R_WSDOT),
                            rhs=_r(ths[a][:], USE_F32R_WSDOT),
                            start=(a == 0),
                            stop=(a == AT - 1),
                        )
                    scst = stagepool.tile([1, SC_CH], F32, tag="scst")
                    nc.scalar.activation(
                        scst[:], pssc[:],
                        mybir.ActivationFunctionType.Identity,
                        bias=bs_sb[:, 0:1],
                    )
                    # repartition (1, 400) -> (2, 200)
                    nc.sync.dma_start(out=sc16[2 * ch:2 * ch + 2, :], in_=scst[:])

                # -- mask, write scores, argmax --
                masked = stpool.tile([D, L], F32, tag="masked")
                nc.vector.select(masked[:], accmask[:], negt[:], sc16[:])
                nc.sync.dma_start(out=d_scores[:, t, :], in_=masked[:])

                mx8 = stagepool.tile([D, 8], F32, tag="mx8")
                nc.vector.max(mx8[:], masked[:])
                ix8 = stagepool.tile([D, 8], U32, tag="ix8")
                nc.vector.max_index(ix8[:], mx8[:], masked[:])
                idx32 = stpool.tile([D, 1], I32, tag="idx32")
                nc.vector.tensor_copy(idx32[:], ix8[:, 0:1])
                nc.sync.dma_start(out=d_sel[:, t:t + 1], in_=idx32[:])

                if t < T - 1:
                    idxf = stagepool.tile([D, 1], F32, tag="idxf")
                    nc.vector.tensor_copy(idxf[:], ix8[:, 0:1])
                    oh = stagepool.tile([D, L], U8, tag="oh", bufs=1)
                    nc.vector.tensor_scalar(
                        out=oh[:], in0=iota_sb[:], scalar1=idxf[:, 0:1],
                        scalar2=None, op0=mybir.AluOpType.is_equal,
                    )
                    am = stpool.tile([D, L], U8, tag="accmask")
                    nc.vector.tensor_max(am[:], accmask[:], oh[:])
                    accmask = am
                    fi = stpool.tile([D, 1], I32, tag="fi")
                    nc.vector.tensor_add(fi[:], docoff_sb[:], idx32[:])
                    flat_idx = fi

    nc.compile()
    return nc


_CACHED_NC = None


def _get_nc():
    global _CACHED_NC
    if _CACHED_NC is None:
        _CACHED_NC = build_program()
    return _CACHED_NC


def _pack_rep(v, nt):
    """(nt*128,) feature vector -> (128, nt*D) broadcast-packed over docs."""
    m = v.reshape(nt, 128).T                      # (128, nt)
    return np.ascontiguousarray(np.repeat(m, D, axis=1))


def make_in_maps(inputs):
    x = np.asarray(inputs["sent_enc_doc"], dtype=np.float32)
    h0_w = np.asarray(inputs["h0_w"], dtype=np.float32)
    h0_b = np.asarray(inputs["h0_b"], dtype=np.float32)
    wih = np.asarray(inputs["gru_w_ih"], dtype=np.float32)
    whh = np.asarray(inputs["gru_w_hh"], dtype=np.float32)
    bih = np.asarray(inputs["gru_b_ih"], dtype=np.float32)
    bhh = np.asarray(inputs["gru_b_hh"], dtype=np.float32)
    wq = np.asarray(inputs["wq"], dtype=np.float32)
    bq = np.asarray(inputs["bq"], dtype=np.float32)
    wd = np.asarray(inputs["wd"], dtype=np.float32)
    bd = np.asarray(inputs["bd"], dtype=np.float32)
    ws = np.asarray(inputs["ws"], dtype=np.float32)
    bs = np.asarray(inputs["bs"], dtype=np.float32)

    lastback = x[:, 0, SH:]
    h0 = np.tanh(lastback @ h0_w.T + h0_b).astype(np.float32)   # (B, EH)

    wdT = np.ascontiguousarray(wd.T)
    wihT = np.ascontiguousarray(wih.T)
    whhT = np.ascontiguousarray(whh.T)
    wqT = np.ascontiguousarray(wq.T)
    qb = np.ascontiguousarray((bq + bd).reshape(4, 128).T)       # (128, 4)
    bsum = bih + bhh
    brz = _pack_rep(bsum[:2 * EH], 8)
    bhn = _pack_rep(bhh[2 * EH:], 4)
    bxn = _pack_rep(bih[2 * EH:], 4)
    wsk = np.ascontiguousarray(ws[0].reshape(4, 128).T)          # (128, 4)
    bs_t = bs.reshape(1, 1)
    iota = np.ascontiguousarray(
        np.broadcast_to(np.arange(L, dtype=np.float32), (D, L)))
    docoff = (np.arange(D, dtype=np.int32) * L).reshape(D, 1)

    in_maps = []
    for c in range(NCORES):
        xc = x[c * D:(c + 1) * D].reshape(ML, D2)
        h0c = h0[c * D:(c + 1) * D]                               # (D, EH)
        h0pk = np.ascontiguousarray(
            h0c.reshape(D, 4, 128).transpose(2, 1, 0).reshape(128, 4 * D))
        in_maps.append({
            "xT": np.ascontiguousarray(xc.T),
            "xnat": np.ascontiguousarray(xc),
            "wdT": wdT, "wihT": wihT, "whhT": whhT, "wqT": wqT,
            "h0pk": h0pk, "qb": qb, "brz": brz, "bhn": bhn, "bxn": bxn,
            "wsk": wsk, "bs": bs_t, "iota": iota, "docoff": docoff,
        })
    return in_maps


def kernel(**inputs):
    nc = _get_nc()
    in_maps = make_in_maps(inputs)
    trace = os.environ.get("KERNEL_TRACE", "0") == "1"
    res = bass_utils.run_bass_kernel_spmd(
        nc, in_maps, core_ids=list(range(NCORES)), trace=trace,
    )
    kernel.last_results = res
    scores = np.concatenate(
        [res.results[c]["scores_out"] for c in range(NCORES)], axis=0)
    sel = np.concatenate(
        [res.results[c]["sel_out"] for c in range(NCORES)], axis=0)
    return scores, sel.astype(np.int32)


kernel.last_results = None
